# revision 40
# baseline (speedup 1.0000x reference)
"""Event-driven FFN kernel for Trainium2 (8 NeuronCores, data-parallel).

Reference computation (per row r of x[32768, 512]):
    mask[r] = any(|x[r, :]| > 0.01)
    y[r, :] = mask[r] * (relu(x[r, :] @ w1 + b1) @ w2 + b2)

Sharding: rows (B*T*S = 32768) split evenly across 8 cores; FFN weights
replicated.  Per core: 4096 rows, processed in 8 blocks of 512 rows.

Per-block dataflow on one core (512 rows per block):
  - DMA x block natural layout [128p, 4rs, 512d]
  - abs-max over d per row -> spike mask (VectorE reduce + is_gt)
  - PE transpose (identity matmul, f32r) -> xT [128d_in, 4dc, 512r]
  - mm1 per f-chunk (16): psum_h[f,r] += w1[dc,f].T @ xT[dc,r]  (4 MMs)
    ReLU+b1 on ScalarE -> hT sbuf [128f_in, 16fc, 512r] (f32r, rounded)
  - mm2 one f-chunk behind mm1 (software pipeline): psum_y[rt] +=
    hT[:,fc,rt].T @ w2[fc,:] -> natural-layout y rows in PSUM (4 banks
    live across the f loop; 5-slot pool so slot reuse never stalls PE)
  - epilogue: yb = b2*mask precomputed off-path; one fused VectorE op per
    row-subtile yout = psy*mask + yb, then DMA out per row-subtile

Scheduling notes:
  - Block b+1's x load + PE transposes are emitted mid-way through block
    b's f-loop so PE never stalls on the DVE xT copies.
  - Weights stream in chunks in first-use order; block 0 defers all mm2s
    past its mm1 phase so they aren't gated on the still-streaming w2.
  - All matmuls use float32r (PE truncates to ~FP22, full 1 cycle/row
    streaming rate; plain float32 runs 4 passes = 4x slower).  Rel err vs
    the f32 reference is ~2e-4.
  - Built on bacc.Bacc: finalize() legalizes multi-sem-wait instructions
    (TRN2 engines accept one sem wait per instruction).

TimelineSim (cost model): ~248 us/core end-to-end, PE 92.7% busy
(pure matmul floor is ~218 us + 10 us transposes).
"""

import numpy as np

N_CORES = 8
ROWS_TOTAL = 32768  # 4 * 16 * 512
ROWS_PER_CORE = ROWS_TOTAL // N_CORES  # 4096
D = 512
F = 2048
R_BLOCK = 512
N_BLOCKS = ROWS_PER_CORE // R_BLOCK  # 8
P = 128
DC = D // P  # 4 d-chunks
FC = F // P  # 16 f-chunks
RT = R_BLOCK // P  # 4 row-subtiles per block
THRESHOLD = 0.01

_CACHE = {}


def _build_program(repeat=1):
    import concourse.mybir as mybir
    import concourse.tile as tile
    from concourse import bacc
    from concourse.masks import make_identity

    f32 = mybir.dt.float32
    f32r = mybir.dt.float32r
    # Bacc (not plain Bass): finalize() runs the wait-splitting legalization
    # (generate_event_semaphores) required by TRN2's 1-wait-per-instruction
    # hardware limit.
    nc = bacc.Bacc()

    x = nc.declare_dram_parameter("x", [ROWS_PER_CORE, D], f32, isOutput=False)
    w1 = nc.declare_dram_parameter("w1", [D, F], f32, isOutput=False)
    b1 = nc.declare_dram_parameter("b1", [F], f32, isOutput=False)
    w2 = nc.declare_dram_parameter("w2", [F, D], f32, isOutput=False)
    b2 = nc.declare_dram_parameter("b2", [D], f32, isOutput=False)
    y = nc.declare_dram_parameter("y", [ROWS_PER_CORE, D], f32, isOutput=True)

    n_iter = N_BLOCKS * repeat

    with tile.TileContext(nc) as tc:
        with (
            tc.tile_pool(name="const", bufs=1) as const,
            tc.tile_pool(name="xin", bufs=2) as xin_pool,
            tc.tile_pool(name="xt", bufs=2) as xt_pool,
            tc.tile_pool(name="h", bufs=2) as h_pool,
            tc.tile_pool(name="out", bufs=2) as out_pool,
            tc.tile_pool(name="mask", bufs=2) as mask_pool,
            tc.tile_pool(name="stage", bufs=2, space="PSUM") as stage_pool,
            tc.tile_pool(name="py", bufs=5, space="PSUM") as py_pool,
            tc.tile_pool(name="warm", bufs=1, space="PSUM") as warm_pool,
        ):
            # Replicated parameters.  Chunked so the first matmuls can start
            # as soon as their slice arrives instead of behind 8 MB of DMA.
            w1s = const.tile([P, DC, F], f32r)  # [p, dc, f] <- w1[dc*128+p, f]
            w2s = const.tile([P, FC, D], f32r)  # [p, fc, d] <- w2[fc*128+p, d]
            b1s = const.tile([P, FC], f32)  # [p, fc] <- b1[fc*128+p]
            b2s = const.tile([P, D], f32)  # b2 replicated to all partitions
            ident = const.tile([P, P], f32r)

            w1r = w1.rearrange("(dc p) f -> p dc f", p=P).bitcast(f32r)
            w2r = w2.rearrange("(fc p) d -> p fc d", p=P).bitcast(f32r)

            def load_x(blk, halves=1):
                rows = x[blk * R_BLOCK : (blk + 1) * R_BLOCK, :]
                src_ap = rows.rearrange("(rs p) d -> p rs d", p=P).bitcast(f32r)
                xn = xin_pool.tile([P, RT, D], f32r, name="xn")
                if halves == 1:
                    nc.sync.dma_start(xn[:], src_ap)
                else:
                    nc.sync.dma_start(xn[:, 0:2, :], src_ap[:, 0:2, :])
                    nc.sync.dma_start(xn[:, 2:4, :], src_ap[:, 2:4, :])
                return xn

            def mask_and_transpose(xn):
                # Spike mask: 1.0 where max_d |x| > threshold else 0.0.
                amax = mask_pool.tile([P, RT], f32, name="amax")
                nc.vector.tensor_reduce(
                    amax[:],
                    xn.bitcast(f32)[:],
                    axis=mybir.AxisListType.X,
                    op=mybir.AluOpType.max,
                    apply_absolute_value=True,
                )
                mask = mask_pool.tile([P, RT], f32, name="mask")
                nc.vector.tensor_scalar(
                    mask[:], amax[:], THRESHOLD, None, op0=mybir.AluOpType.is_gt
                )

                # Transpose x -> xT [d_inner, dc, r] via PE (f32r: 1.5 c/row).
                xT = xt_pool.tile([P, DC, R_BLOCK], f32r, name="xT")
                for rs in range(RT):
                    pt = stage_pool.tile(
                        [P, DC, P], f32r, name="pt", tag="stage"
                    )
                    for dc in range(DC):
                        nc.tensor.transpose(
                            pt[:, dc, :],
                            xn[:, rs, dc * P : (dc + 1) * P],
                            ident[:],
                        )
                    nc.vector.tensor_copy(xT[:, :, rs * P : (rs + 1) * P], pt[:])
                return {"xT": xT, "mask": mask}

            # PE clock warm-up: the PE ramps to full clock only after ~3us
            # of sustained activity (HAM gate).  Burn the ramp on
            # dependency-free dummy matmuls (memset-fed, bf16) during the
            # dead window while x block 0 / w1 stream in, so the real
            # transposes+matmuls start at full rate.
            bf16 = mybir.dt.bfloat16
            wsrc = const.tile([P, D], bf16)
            nc.vector.memset(wsrc[:], 0.0)
            wdummy = warm_pool.tile([P, D], f32)
            for _ in range(9):
                nc.tensor.matmul(
                    wdummy[:], wsrc[:, 0:P], wsrc[:], start=True, stop=True
                )

            # --- startup: stream in first-use order.  Block 0's mm1 phase
            # only needs w1 (streamed in quarters just ahead of use); w2
            # chunks follow and land before block 0's (deferred) mm2 phase.
            xn0 = load_x(0)
            nc.sync.dma_start(w1s[:, :, 0:512], w1r[:, :, 0:512])
            nc.sync.dma_start(b1s[:], b1.rearrange("(fc p) -> p fc", p=P))
            nc.sync.dma_start(w1s[:, :, 512:1024], w1r[:, :, 512:1024])
            # Build identity in f32 scratch, then copy (=round) into the
            # f32r tile the transposes consume (BIR verifier requirement).
            ident_f32 = const.tile([P, P], f32)
            make_identity(nc, ident_f32)
            nc.vector.tensor_copy(ident[:], ident_f32[:])
            cur = mask_and_transpose(xn0)
            # Absorbs the w1-chunk-0 DMA wait onto a throwaway matmul.
            nc.tensor.matmul(
                wdummy[:, 0:P],
                w1s[:, 0, 0:P].bitcast(f32),
                w1s[:, 0, 0:P].bitcast(f32),
                start=True,
                stop=True,
            )
            nc.sync.dma_start(w1s[:, :, 1024:1536], w1r[:, :, 1024:1536])
            nc.sync.dma_start(w1s[:, :, 1536:2048], w1r[:, :, 1536:2048])
            # x(1) right after w1 (its transposes run early in block 0's
            # deferred-mm2 phase), then w2 chunks just ahead of their mm2s.
            xn_next = load_x(1 % N_BLOCKS) if n_iter > 1 else None
            for wc in range(4):
                nc.sync.dma_start(
                    w2s[:, 4 * wc : 4 * (wc + 1), :],
                    w2r[:, 4 * wc : 4 * (wc + 1), :],
                )
            nc.sync.dma_start(b2s[:], b2[None, :].to_broadcast([P, D]))

            for it in range(n_iter):
                blk = it % N_BLOCKS
                xT, mask = cur["xT"], cur["mask"]

                hs = h_pool.tile([P, FC, R_BLOCK], f32r, name="hs")  # h^T
                psy = [
                    py_pool.tile([P, D], f32, name=f"psy{rt}", tag="psy")
                    for rt in range(RT)
                ]
                nxt = None
                # b2 * mask per row-subtile, off the critical path (feeds
                # the fused single-op epilogue below).
                yb = out_pool.tile([P, RT, D], f32, name="yb")
                for rt in range(RT):
                    nc.vector.tensor_scalar_mul(
                        yb[:, rt, :], b2s[:], mask[:, rt : rt + 1]
                    )

                def mm2(fc):
                    for rt in range(RT):
                        nc.tensor.matmul(
                            psy[rt][:],
                            hs[:, fc, rt * P : (rt + 1) * P],
                            w2s[:, fc, :],
                            start=(fc == 0),
                            stop=(fc == FC - 1),
                        )

                # Software-pipelined: mm2 runs one f-chunk behind mm1/relu
                # so PE never waits on ScalarE at block boundaries.  Block 0
                # instead defers ALL mm2s past the mm1 phase so they aren't
                # stuck behind the still-streaming w2 (PE does w1-only work
                # while w2 lands).
                mm2_lag = FC if it == 0 else 2
                for fc in range(FC):
                    ph = stage_pool.tile(
                        [P, R_BLOCK], f32, name="ph", tag="stage"
                    )
                    for dc in range(DC):
                        nc.tensor.matmul(
                            ph[:],
                            w1s[:, dc, fc * P : (fc + 1) * P],
                            xT[:, dc, :],
                            start=(dc == 0),
                            stop=(dc == DC - 1),
                        )
                    nc.scalar.activation(
                        hs[:, fc, :],
                        ph[:],
                        mybir.ActivationFunctionType.Relu,
                        bias=b1s[:, fc : fc + 1],
                    )
                    if fc >= mm2_lag:
                        mm2(fc - mm2_lag)
                    # Prefetch: x DMA for block it+2 early (fc==1), next
                    # block's transposes mid-way so PE never stalls.  For
                    # block 0 the transposes wait until fc==15 (x(1) is still
                    # behind w1 in the DMA stream at fc==7).
                    if fc == 1 and it + 2 < n_iter:
                        xn_next2 = load_x((it + 2) % N_BLOCKS)
                    if fc == (15 if it == 0 else 7) and it + 1 < n_iter:
                        nxt = mask_and_transpose(xn_next)
                        xn_next = xn_next2 if it + 2 < n_iter else None
                for fc in range(FC - mm2_lag, FC):
                    mm2(fc)

                # Epilogue: yout = psy*mask + b2*mask, one fused DVE op per
                # row-subtile (psy bank freed after a single 658ns op).
                yout = out_pool.tile([P, RT, D], f32, name="yout")
                for rt in range(RT):
                    nc.vector.scalar_tensor_tensor(
                        yout[:, rt, :],
                        psy[rt][:],
                        mask[:, rt : rt + 1],
                        yb[:, rt, :],
                        op0=mybir.AluOpType.mult,
                        op1=mybir.AluOpType.add,
                    )
                    out_rows = y[
                        blk * R_BLOCK + rt * P : blk * R_BLOCK + (rt + 1) * P, :
                    ]
                    nc.sync.dma_start(out_rows, yout[:, rt, :])
                cur = nxt

    nc.finalize()
    return nc


def _get_program():
    if "nc" not in _CACHE:
        _CACHE["nc"] = _build_program()
    return _CACHE["nc"]


def kernel(x, w1, b1, w2, b2, _trace=False):
    from concourse.bass_utils import run_bass_kernel_spmd

    x = np.ascontiguousarray(np.asarray(x, dtype=np.float32))
    w1 = np.ascontiguousarray(np.asarray(w1, dtype=np.float32))
    b1 = np.ascontiguousarray(np.asarray(b1, dtype=np.float32))
    w2 = np.ascontiguousarray(np.asarray(w2, dtype=np.float32))
    b2 = np.ascontiguousarray(np.asarray(b2, dtype=np.float32))

    B, T, S, Dd = x.shape
    xf = x.reshape(-1, Dd)
    shards = np.split(xf, N_CORES, axis=0)
    in_maps = [
        {"x": s, "w1": w1, "b1": b1, "w2": w2, "b2": b2} for s in shards
    ]

    nc = _get_program()
    res = run_bass_kernel_spmd(nc, in_maps, list(range(N_CORES)), trace=_trace)
    yf = np.concatenate([r["y"] for r in res.results], axis=0)
    out = yf.reshape(B, T, S, Dd).astype(np.float32)
    if _trace:
        return out, res
    return out
